# revision 5
# baseline (speedup 1.0000x reference)
"""Two-layer Elman RNN (B=64, S=512, EMB=512, HID=1024) on 8 TRN2 NeuronCores.

Fully data-parallel: each core owns 8 batch rows and runs BOTH layer scans
locally — no collectives at all. Activations are kept feature-major
([128 feature-partition, (m, b) columns]) so each scan step is an 8x8 grid of
[128,128] fp16 matmuls with batch (8 cols) streaming, plus one identity
matmul that injects the precomputed input transform into the PSUM
accumulation, and a single [128, 64] Tanh per layer per step.

Schedule (per core): the layer-1 chain act1(t) -> psum1(t+1) -> act1(t+1) is
the critical path (~1us/step in the timeline model: act visible latency +
64-matmul grid + PSUM drain). Everything else is arranged as always-ready
filler that the Tile list-scheduler packs into the chain's latency windows:
  - embedding gather + PE transpose + pre1 = xe@Wi1 + b1, in 16-step chunks
    emitted two chunks ahead;
  - pre2 = h1@Wi2 + b2 computed in bulk from the h1 archive in 8-step
    sub-chunks (layer 2 runs LAG steps behind layer 1, so only the small
    Wh2 grid and act2 remain near the chain);
  - weights arrive pre-shuffled from the host as single-DMA [128, 8192]
    tiles (one DMACopy each, issued on different queues).
Matmuls in fp16 (fp32 PSUM), tanh/sigmoid in fp32->fp16. fp8 was measured
numerically and rejected: the 512-step recurrence amplifies e4m3 weight /
activation quantization noise to ~10-25% output error (fp16: ~2e-3).
"""

from contextlib import ExitStack

import numpy as np

import concourse.bass as bass
import concourse.bacc as bacc
import concourse.mybir as mybir
import concourse.tile as tile
from concourse.bass import IndirectOffsetOnAxis
from concourse.bass_utils import run_bass_kernel_spmd
from concourse.masks import make_identity

P = 128
VOCAB, EMB, HID = 50257, 512, 1024
B, S = 64, 512
NCORES = 8
BL = B // NCORES          # 8 batch rows per core
M = HID // P              # 8 output feature chunks
KH = HID // P             # 8 contraction chunks (hidden)
KE = EMB // P             # 4 contraction chunks (embedding)
W = M * BL                # 64 scan columns per step (m, b)
EC = 16                   # embed/pre1 chunk: 16 steps = 128 tokens
NEC = S // EC             # 32 chunks
SC = 8                    # pre2 bulk sub-chunk steps
LAG = 10                  # layer-2 lag in steps

F32 = mybir.dt.float32
F16 = mybir.dt.float16
I32 = mybir.dt.int32
NP16 = np.float16

_BUILT = {}


def build(local_cc=False):
    # local_cc is accepted for test-harness compatibility; this kernel has
    # no collectives so the flag changes nothing.
    nc = bacc.Bacc("TRN2", target_bir_lowering=False, debug=False,
                   num_devices=NCORES)

    xg_d = nc.dram_tensor("xg", [S * BL // P, P], I32, kind="ExternalInput").ap()
    emb_d = nc.dram_tensor("emb", [VOCAB, EMB], F16, kind="ExternalInput").ap()
    wi1_d = nc.dram_tensor("wi1l", [P, KE * M * P], F16, kind="ExternalInput").ap()
    wh1_d = nc.dram_tensor("wh1l", [P, KH * M * P], F16, kind="ExternalInput").ap()
    wh2_d = nc.dram_tensor("wh2l", [P, KH * M * P], F16, kind="ExternalInput").ap()
    wi2_d = nc.dram_tensor("wi2l", [P, KH * M * P], F16, kind="ExternalInput").ap()
    b1_d = nc.dram_tensor("b1m", [M, P], F32, kind="ExternalInput").ap()
    b2_d = nc.dram_tensor("b2m", [M, P], F32, kind="ExternalInput").ap()
    wd_d = nc.dram_tensor("wdk", [P, KH], F16, kind="ExternalInput").ap()
    bd_d = nc.dram_tensor("bdv", [BL], F32, kind="ExternalInput").ap()
    y_d = nc.dram_tensor("y", [BL], F32, kind="ExternalOutput").ap()

    AF = mybir.ActivationFunctionType

    with tile.TileContext(nc) as tc, ExitStack() as ctx:
        cpool = ctx.enter_context(tc.tile_pool(name="const", bufs=1))
        wpool = ctx.enter_context(tc.tile_pool(name="w", bufs=1))
        prepool = ctx.enter_context(tc.tile_pool(name="pre1", bufs=1))
        gpool = ctx.enter_context(tc.tile_pool(name="gather", bufs=2))
        xtpool = ctx.enter_context(tc.tile_pool(name="xet", bufs=2))
        a1pool = ctx.enter_context(tc.tile_pool(name="arch1", bufs=2 * SC + LAG + 6))
        a2pool = ctx.enter_context(tc.tile_pool(name="arch2", bufs=6))
        p2pool = ctx.enter_context(tc.tile_pool(name="pre2", bufs=3))
        pp1 = ctx.enter_context(tc.tile_pool(name="ps1", bufs=2, space="PSUM"))
        pp2 = ctx.enter_context(tc.tile_pool(name="ps2", bufs=2, space="PSUM"))
        ppb = ctx.enter_context(tc.tile_pool(name="psb", bufs=1, space="PSUM"))
        ppe = ctx.enter_context(tc.tile_pool(name="pse", bufs=1, space="PSUM"))

        ident = cpool.tile([P, P], F16, name="ident")
        make_identity(nc, ident[:])
        b1t = cpool.tile([P, M], F32, name="b1t")
        nc.sync.dma_start(out=b1t[:], in_=b1_d.rearrange("m p -> p m"))
        b2t = cpool.tile([P, M], F32, name="b2t")
        nc.sync.dma_start(out=b2t[:], in_=b2_d.rearrange("m p -> p m"))
        wd_sb = cpool.tile([P, KH], F16, name="wd_sb")
        nc.sync.dma_start(out=wd_sb[:], in_=wd_d[:])
        bd_sb = cpool.tile([P, 1], F32, name="bd_sb")
        nc.sync.dma_start(out=bd_sb[0:BL, 0:1], in_=bd_d[:])

        # weights: single pre-shuffled DMA each, on four different queues so
        # the sequencer-side DMA setup (~650ns each) doesn't serialize
        wh2 = wpool.tile([P, KH * M * P], F16, name="wh2")
        nc.sync.dma_start(out=wh2[:], in_=wh2_d[:])
        wi2 = wpool.tile([P, KH * M * P], F16, name="wi2")
        nc.scalar.dma_start(out=wi2[:], in_=wi2_d[:])
        wh1 = wpool.tile([P, KH * M * P], F16, name="wh1")
        nc.gpsimd.dma_start(out=wh1[:], in_=wh1_d[:])
        wi1 = wpool.tile([P, KE * M * P], F16, name="wi1")
        nc.sync.dma_start(out=wi1[:], in_=wi1_d[:])

        # full input-transform archive PRE1[p, (t, m, b)]
        pre1 = prepool.tile([P, S * W], F16, name="pre1")
        pre1_v = pre1[:].rearrange("p (t mb) -> p t mb", mb=W)

        def phase_a(c):
            """Embed + transpose + pre1(+b1) for steps [c*EC, (c+1)*EC)."""
            idx = gpool.tile([P, 1], I32, tag="idx", name=f"idx{c}")
            nc.sync.dma_start(out=idx[:, 0:1], in_=xg_d[c, :])
            xe_g = gpool.tile([P, EMB], F16, tag="xe", name=f"xe{c}")
            nc.gpsimd.indirect_dma_start(
                out=xe_g[:], out_offset=None, in_=emb_d[:],
                in_offset=IndirectOffsetOnAxis(ap=idx[:, 0:1], axis=0))
            xeT = xtpool.tile([P, EMB], F16, tag="xeT", name=f"xeT{c}")
            for e in range(KE):
                tp = ppe.tile([P, P], F16, tag="tp", name=f"tp{c}_{e}")
                nc.tensor.transpose(out=tp[:], in_=xe_g[:, e * P:(e + 1) * P],
                                    identity=ident[:])
                nc.vector.tensor_copy(out=xeT[:, e * P:(e + 1) * P], in_=tp[:])
            for m in range(M):
                bp = ppb.tile([P, EC * BL], F32, tag="bp1", name=f"bp1{c}_{m}")
                for e in range(KE):
                    nc.tensor.matmul(
                        bp[:], lhsT=wi1[:, (e * M + m) * P:(e * M + m + 1) * P],
                        rhs=xeT[:, e * P:(e + 1) * P],
                        start=(e == 0), stop=(e == KE - 1),
                        skip_group_check=True)
                dst = pre1_v[:, c * EC:(c + 1) * EC, m * BL:(m + 1) * BL]
                nc.vector.tensor_scalar_add(
                    out=dst, in0=bp[:].rearrange("p (t b) -> p t b", b=BL),
                    scalar1=b1t[:, m:m + 1])

        def grid(ps, whs, rhs_tile, last):
            for m in range(M):
                for k in range(KH):
                    nc.tensor.matmul(
                        ps[:, m * BL:(m + 1) * BL],
                        lhsT=whs[:, (k * M + m) * P:(k * M + m + 1) * P],
                        rhs=rhs_tile[:, k * BL:(k + 1) * BL],
                        start=False, stop=(last and k == KH - 1),
                        skip_group_check=True)

        h1, h2, ps1, ps2, pre2 = {}, {}, {}, {}, {}

        h1[-1] = a1pool.tile([P, W], F16, tag="a1", name="h1_m1")
        nc.vector.memset(h1[-1][:], 0.0)
        h2[-2] = a2pool.tile([P, W], F16, tag="a2", name="h2_m2")
        nc.vector.memset(h2[-2][:], 0.0)
        h2[-1] = h2[-2]

        def make_ps1(t):
            ps = pp1.tile([P, W], F32, tag="ps1", name=f"ps1_{t}")
            nc.tensor.matmul(ps[:], lhsT=ident[:],
                             rhs=pre1_v[:, t, :],
                             start=True, stop=False, skip_group_check=True)
            grid(ps, wh1, h1[t - 1], True)
            ps1[t] = ps

        def bulk_pre2(sc):
            """pre2(+b2) for steps [sc*SC, (sc+1)*SC) from the h1 archive."""
            t0 = sc * SC
            p2 = p2pool.tile([P, SC * W], F16, tag="p2", name=f"pre2_{sc}")
            for m in range(M):
                bp = ppb.tile([P, SC * BL], F32, tag="bp2", name=f"bp2{sc}_{m}")
                for i in range(SC):
                    for k in range(KH):
                        nc.tensor.matmul(
                            bp[:, i * BL:(i + 1) * BL],
                            lhsT=wi2[:, (k * M + m) * P:(k * M + m + 1) * P],
                            rhs=h1[t0 + i][:, k * BL:(k + 1) * BL],
                            start=(k == 0), stop=(k == KH - 1),
                            skip_group_check=True)
                dst = p2[:].rearrange("p (i mb) -> p i mb", mb=W)[
                    :, :, m * BL:(m + 1) * BL]
                nc.vector.tensor_scalar_add(
                    out=dst, in0=bp[:].rearrange("p (i b) -> p i b", b=BL),
                    scalar1=b2t[:, m:m + 1])
            pre2[sc] = p2

        def make_ps2(tl):
            ps = pp2.tile([P, W], F32, tag="ps2", name=f"ps2_{tl}")
            p2 = pre2[tl // SC]
            nc.tensor.matmul(
                ps[:], lhsT=ident[:],
                rhs=p2[:, (tl % SC) * W:(tl % SC + 1) * W],
                start=True, stop=False, skip_group_check=True)
            grid(ps, wh2, h2[tl - 1], True)
            ps2[tl] = ps

        def make_act2(ta):
            h2[ta] = a2pool.tile([P, W], F16, tag="a2", name=f"h2_{ta}")
            nc.scalar.activation(out=h2[ta][:], in_=ps2[ta][:], func=AF.Tanh)

        phase_a(0)
        phase_a(1)
        make_ps1(0)
        for t in range(S + LAG + 2):
            if t < S:
                h1[t] = a1pool.tile([P, W], F16, tag="a1", name=f"h1_{t}")
                nc.scalar.activation(out=h1[t][:], in_=ps1[t][:], func=AF.Tanh)
                if t + 1 < S:
                    make_ps1(t + 1)
                if t % EC == 0 and t // EC + 2 < NEC:
                    phase_a(t // EC + 2)
                if t % SC == SC - 1:
                    bulk_pre2(t // SC)
            ta = t - 2 - LAG
            if ta >= 0:
                make_act2(ta)
            tl = t - 1 - LAG
            if 0 <= tl < S:
                make_ps2(tl)
            h1.pop(t - 2 * SC - LAG - 4, None)
            h2.pop(t - LAG - 8, None)
            ps1.pop(t - 2, None)
            ps2.pop(t - 4 - LAG, None)
            pre2.pop(t // SC - 3, None)

        # head: y = sigmoid(h2_last @ Wd + bd)
        with tc.tile_pool(name="hps", bufs=1, space="PSUM") as hpool:
            hps = hpool.tile([BL, 1], F32, name="hps")
            last = h2[S - 1]
            for k in range(KH):
                nc.tensor.matmul(
                    hps[:], lhsT=last[:, k * BL:(k + 1) * BL],
                    rhs=wd_sb[:, k:k + 1], start=(k == 0), stop=(k == KH - 1))
            y_sb = cpool.tile([P, 1], F32, name="y_sb")
            nc.scalar.activation(out=y_sb[0:BL, 0:1], in_=hps[:],
                                 func=AF.Sigmoid, bias=bd_sb[0:BL, 0:1])
            nc.sync.dma_start(out=y_d[:], in_=y_sb[0:BL, 0:1])

    nc.compile()
    return nc


def _shuffle_w(w, kc):
    """[kc*128, 1024] -> [128, kc*8*128] with layout (p, (k, m, q))."""
    return np.ascontiguousarray(
        np.asarray(w, NP16).reshape(kc, P, M, P).transpose(1, 0, 2, 3)
        .reshape(P, kc * M * P))


def _prep_maps(x, emb, Wi1, Wh1, b1, Wi2, Wh2, b2, Wd, bd):
    x = np.asarray(x, np.int32)
    shared = {
        "emb": np.ascontiguousarray(np.asarray(emb, NP16)),
        "wi1l": _shuffle_w(Wi1, KE),
        "wh1l": _shuffle_w(Wh1, KH),
        "wh2l": _shuffle_w(Wh2, KH),
        "wi2l": _shuffle_w(Wi2, KH),
        "b1m": np.ascontiguousarray(np.asarray(b1, np.float32).reshape(M, P)),
        "b2m": np.ascontiguousarray(np.asarray(b2, np.float32).reshape(M, P)),
        "wdk": np.ascontiguousarray(
            np.asarray(Wd, NP16).reshape(KH, P).T),
        "bdv": np.ascontiguousarray(
            np.broadcast_to(np.asarray(bd, np.float32), (BL,))),
    }
    in_maps = []
    for c in range(NCORES):
        xs = x[c * BL:(c + 1) * BL, :]                    # [8, 512]
        xgrp = np.ascontiguousarray(xs.T).reshape(-1, P)  # (t, b) order
        in_maps.append({**shared, "xg": xgrp})
    return in_maps


def kernel(x, emb, Wi1, Wh1, b1, Wi2, Wh2, b2, Wd, bd):
    if "nc" not in _BUILT:
        _BUILT["nc"] = build()
    nc = _BUILT["nc"]
    in_maps = _prep_maps(x, emb, Wi1, Wh1, b1, Wi2, Wh2, b2, Wd, bd)
    res = run_bass_kernel_spmd(nc, in_maps, list(range(NCORES)))
    kernel.last_result = res
    y = np.concatenate([np.asarray(res.results[c]["y"], np.float32)
                        for c in range(NCORES)])
    return y
